# revision 10
# baseline (speedup 1.0000x reference)
"""nn_ConcatAttention TRN2 Bass kernel.

Data-parallel over batch: 8 NeuronCores x 4 batches each. Per core/batch:
  precompute^T[d,s] = sum_a W_pre[d,a]*context[s,a] + b_pre[d]   (PE f32r, ACT bias)
  tmp = tanh(precompute^T + (b_pre + W_q@input)[d])              (ACT, fused bias)
  energy[s] = sum_d v[d]*tmp[d,s]                                (PE matvec)
  escore = exp(energy + mask_addend)        (softmax w/o max-sub; masked -> 0)
  wc[a] = sum_s escore[s]*context[s,a]      (DVE TTR vs PE-broadcast scores)
  score = escore / sum(escore)
Host: pre-transposes context/W, transposes precompute^T back, normalizes wc.
"""
import os as _os
import numpy as np
from contextlib import ExitStack

import concourse.mybir as mybir
import concourse.tile as tile
from concourse import bacc
from concourse.bass_utils import run_bass_kernel_spmd

B, S, A, Q, D = 32, 2048, 1024, 1024, 1024
NCORES = 8
BLOC = B // NCORES          # batches per core
SC = 512                    # s-chunk (PSUM bank width in fp32)
NSC = S // SC
NA = A // 128
ND = D // 128
NQ = Q // 128

f32 = mybir.dt.float32
f32r = mybir.dt.float32r
AF = mybir.ActivationFunctionType
ALU = mybir.AluOpType
AX = mybir.AxisListType

_PROG = None
KSTAGE = int(_os.environ.get("KSTAGE", "5"))


def _build_program():
    nc = bacc.Bacc("TRN2", target_bir_lowering=False, debug=False)

    ctxT_d = nc.dram_tensor("ctxT", [BLOC, A, S], f32r, kind="ExternalInput").ap()
    wpT_d = nc.dram_tensor("wpT", [A, D], f32r, kind="ExternalInput").ap()
    wqT_d = nc.dram_tensor("wqT", [Q, D], f32r, kind="ExternalInput").ap()
    inpT_d = nc.dram_tensor("inpT", [Q, BLOC], f32r, kind="ExternalInput").ap()
    bpre_d = nc.dram_tensor("bpre", [D], f32, kind="ExternalInput").ap()
    v_d = nc.dram_tensor("v", [D], f32r, kind="ExternalInput").ap()
    addend_d = nc.dram_tensor("addend", [BLOC, S], f32, kind="ExternalInput").ap()

    preT_d = nc.dram_tensor("preT", [BLOC, D, S], f32, kind="ExternalOutput").ap()
    score_d = nc.dram_tensor("score", [BLOC, S], f32, kind="ExternalOutput").ap()
    wc_d = nc.dram_tensor("wc", [BLOC, A], f32, kind="ExternalOutput").ap()
    sums_d = nc.dram_tensor("sums", [BLOC], f32, kind="ExternalOutput").ap()

    wc_r = wc_d.rearrange("b (t p) -> b p t", p=128)  # [BLOC, 128, NA]

    with tile.TileContext(nc) as tc, ExitStack() as ctx:
        const_pool = ctx.enter_context(tc.tile_pool(name="const", bufs=1))
        wp_pool = ctx.enter_context(tc.tile_pool(name="wp", bufs=1))

        ones_1 = const_pool.tile([1, 128], f32r)
        nc.vector.memset(ones_1[:].bitcast(f32), 1.0)
        bpre_sb = const_pool.tile([128, ND], f32)
        nc.sync.dma_start(bpre_sb[:], bpre_d.rearrange("(t p) -> p t", p=128))
        v_sb = const_pool.tile([128, ND], f32r)
        nc.sync.dma_start(v_sb[:], v_d.rearrange("(t p) -> p t", p=128))
        qb_sb = const_pool.tile([128, ND, BLOC], f32)

        wpT_sb = []
        for at in range(NA):
            t = wp_pool.tile([128, D], f32r, tag=f"wp{at}")
            nc.sync.dma_start(t[:], wpT_d[at * 128:(at + 1) * 128, :])
            wpT_sb.append(t)

        # ---- q bias: qb[d, b] = b_pre[d] + sum_c W_q[d,c] * input[b,c]
        if KSTAGE >= 2:
            with tc.tile_pool(name="wq", bufs=1) as wq_pool, \
                 tc.tile_pool(name="qpsum", bufs=2, space="PSUM") as q_psum_pool:
                inpT_sb = wq_pool.tile([128, NQ, BLOC], f32r)
                nc.sync.dma_start(inpT_sb[:], inpT_d.rearrange("(t p) b -> p t b", p=128))
                wqT_sb = []
                for ct in range(NQ):
                    t = wq_pool.tile([128, D], f32r, tag=f"wq{ct}")
                    nc.sync.dma_start(t[:], wqT_d[ct * 128:(ct + 1) * 128, :])
                    wqT_sb.append(t)
                for dt in range(ND):
                    qp = q_psum_pool.tile([128, BLOC], f32, tag="qp")
                    for ct in range(NQ):
                        nc.tensor.matmul(qp[:], wqT_sb[ct][:, dt * 128:(dt + 1) * 128],
                                         inpT_sb[:, ct, :], start=(ct == 0), stop=(ct == NQ - 1))
                    nc.vector.tensor_scalar_add(qb_sb[:, dt, :], qp[:], bpre_sb[:, dt:dt + 1])

        # ---- streaming pools
        ctx_pool = ctx.enter_context(tc.tile_pool(name="ctx", bufs=24))
        pre_pool = ctx.enter_context(tc.tile_pool(name="pre", bufs=3))
        tmp_pool = ctx.enter_context(tc.tile_pool(name="tmp", bufs=ND + 2))
        scr_pool = ctx.enter_context(tc.tile_pool(name="scr", bufs=3))
        row_pool = ctx.enter_context(tc.tile_pool(name="rows", bufs=2))
        er_pool = ctx.enter_context(tc.tile_pool(name="er", bufs=2))
        small_pool = ctx.enter_context(tc.tile_pool(name="small", bufs=4))
        wc_pool = ctx.enter_context(tc.tile_pool(name="wcp", bufs=6))
        mm_psum_pool = ctx.enter_context(tc.tile_pool(name="mmp", bufs=2, space="PSUM"))
        e_psum_pool = ctx.enter_context(tc.tile_pool(name="ep", bufs=2, space="PSUM"))
        bc_psum_pool = ctx.enter_context(tc.tile_pool(name="bcp", bufs=2, space="PSUM"))

        for b in range(BLOC):
            if KSTAGE >= 2:
                addend_sb = row_pool.tile([1, S], f32, tag="addend")
                nc.sync.dma_start(addend_sb[:], addend_d[b:b + 1, :])
            if KSTAGE >= 3:
                escore_row = er_pool.tile([1, S], f32r, tag="escore")
                partials = small_pool.tile([1, NSC], f32, tag="partials")
            wc_parts = []

            for sc in range(NSC):
                cts = []
                for at in range(NA):
                    t = ctx_pool.tile([128, SC], f32r, tag="ctx")
                    nc.sync.dma_start(
                        t[:], ctxT_d[b, at * 128:(at + 1) * 128, sc * SC:(sc + 1) * SC])
                    cts.append(t)

                tmps = []
                for dt in range(ND):
                    mp = mm_psum_pool.tile([128, SC], f32, tag="mm")
                    for at in range(NA):
                        nc.tensor.matmul(mp[:], wpT_sb[at][:, dt * 128:(dt + 1) * 128],
                                         cts[at][:], start=(at == 0), stop=(at == NA - 1))
                    pre_sb = pre_pool.tile([128, SC], f32, tag="pre")
                    nc.scalar.activation(pre_sb[:], mp[:], AF.Identity,
                                         bias=bpre_sb[:, dt:dt + 1])
                    nc.sync.dma_start(
                        preT_d[b, dt * 128:(dt + 1) * 128, sc * SC:(sc + 1) * SC], pre_sb[:])
                    if KSTAGE >= 2:
                        tr = tmp_pool.tile([128, SC], f32r, tag="tmp")
                        nc.scalar.activation(tr[:], mp[:], AF.Tanh, bias=qb_sb[:, dt, b:b + 1])
                        tmps.append(tr)

                if KSTAGE >= 2:
                    e_ps = e_psum_pool.tile([1, SC], f32, tag="e")
                    for dt in range(ND):
                        nc.tensor.matmul(e_ps[:], v_sb[:, dt:dt + 1], tmps[dt][:],
                                         start=(dt == 0), stop=(dt == ND - 1))
                    em = row_pool.tile([1, SC], f32, tag="em")
                    nc.vector.tensor_add(em[:], e_ps[:], addend_sb[:, sc * SC:(sc + 1) * SC])
                if KSTAGE == 2:
                    nc.sync.dma_start(score_d[b:b + 1, sc * SC:(sc + 1) * SC], em[:])
                if KSTAGE >= 3:
                    esl = escore_row[:, sc * SC:(sc + 1) * SC]
                    nc.scalar.activation(esl, em[:], AF.Exp)
                    nc.vector.tensor_reduce(partials[:, sc:sc + 1], esl.bitcast(f32),
                                            axis=AX.X, op=ALU.add)
                if KSTAGE >= 4:
                    bc = bc_psum_pool.tile([128, SC], f32, tag="bc")
                    nc.tensor.matmul(bc[:], ones_1[:], esl, start=True, stop=True)
                if KSTAGE == 4 and sc == 0:
                    dbg = scr_pool.tile([1, SC], f32, tag="dbg")
                    nc.vector.tensor_copy(dbg[:], bc[0:1, :])
                    nc.sync.dma_start(wc_d[b:b + 1, 0:SC], dbg[:])
                if KSTAGE >= 5:
                    bc_sb = scr_pool.tile([128, SC], f32, tag="bcs")
                    nc.scalar.copy(bc_sb[:], bc[:])
                    wc_new = wc_pool.tile([128, NA], f32, tag="wc")
                    for at in range(NA):
                        scr = scr_pool.tile([128, SC], f32, tag="scr")
                        nc.vector.tensor_tensor(scr[:], cts[at][:].bitcast(f32),
                                                bc_sb[:], op=ALU.mult)
                        nc.vector.tensor_reduce(wc_new[:, at:at + 1], scr[:],
                                                axis=AX.X, op=ALU.add)
                    wc_parts.append(wc_new)

            if KSTAGE >= 3:
                total = small_pool.tile([1, 1], f32, tag="total")
                nc.vector.tensor_reduce(total[:], partials[:], axis=AX.X, op=ALU.add)
                inv = small_pool.tile([1, 1], f32, tag="inv")
                nc.vector.reciprocal(inv[:], total[:])
                score_sb = row_pool.tile([1, S], f32, tag="score")
                nc.scalar.activation(score_sb[:], escore_row[:].bitcast(f32), AF.Copy,
                                     scale=inv[:])
                nc.sync.dma_start(score_d[b:b + 1, :], score_sb[:])
                nc.sync.dma_start(sums_d[b:b + 1], total[:])
            if KSTAGE >= 5:
                w01 = wc_pool.tile([128, NA], f32, tag="wcs")
                nc.vector.tensor_add(w01[:], wc_parts[0][:], wc_parts[1][:])
                w23 = wc_pool.tile([128, NA], f32, tag="wcs")
                nc.vector.tensor_add(w23[:], wc_parts[2][:], wc_parts[3][:])
                wfin = wc_pool.tile([128, NA], f32, tag="wcf")
                nc.vector.tensor_add(wfin[:], w01[:], w23[:])
                nc.sync.dma_start(wc_r[b], wfin[:])

    nc.compile()
    return nc


def _get_program():
    global _PROG
    if _PROG is None:
        _PROG = _build_program()
    return _PROG


def _make_in_maps(inputs):
    inp = np.ascontiguousarray(np.asarray(inputs["input"], dtype=np.float32))
    context = np.asarray(inputs["context"], dtype=np.float32)
    W_pre = np.asarray(inputs["W_pre"], dtype=np.float32)
    b_pre = np.ascontiguousarray(np.asarray(inputs["b_pre"], dtype=np.float32))
    W_q = np.asarray(inputs["W_q"], dtype=np.float32)
    v = np.ascontiguousarray(np.asarray(inputs["v"], dtype=np.float32))
    mask = np.asarray(inputs["mask"])

    wpT = np.ascontiguousarray(W_pre.T)
    wqT = np.ascontiguousarray(W_q.T)
    addend_full = (mask[:, 0, :].astype(np.float32) - 1.0) * 1e6  # 0 or -1e6

    in_maps = []
    for c in range(NCORES):
        sl = slice(c * BLOC, (c + 1) * BLOC)
        in_maps.append({
            "ctxT": np.ascontiguousarray(context[sl].transpose(0, 2, 1)),
            "wpT": wpT,
            "wqT": wqT,
            "inpT": np.ascontiguousarray(inp[sl, 0, :].T),
            "bpre": b_pre,
            "v": v,
            "addend": np.ascontiguousarray(addend_full[sl]),
        })
    return in_maps


def _assemble(results):
    pre = np.empty((B, S, D), np.float32)
    score = np.empty((B, 1, S), np.float32)
    wc = np.empty((B, 1, A), np.float32)
    for c in range(NCORES):
        r = results[c]
        sums = r["sums"]
        for j in range(BLOC):
            bi = c * BLOC + j
            pre[bi] = r["preT"][j].T
            score[bi, 0] = r["score"][j]
            wc[bi, 0] = r["wc"][j] / sums[j] if KSTAGE >= 5 else r["wc"][j]
    return wc, score, pre


def run_on_hw(inputs, trace=False, **kwargs):
    nc = _get_program()
    in_maps = _make_in_maps(inputs)
    res = run_bass_kernel_spmd(nc, in_maps, list(range(NCORES)), trace=trace, **kwargs)
    return _assemble(res.results), res


def kernel(**inputs):
    (wc, score, pre), _ = run_on_hw(inputs)
    return wc, score, pre
